# revision 16
# baseline (speedup 1.0000x reference)
"""Trainium2 Bass kernel for nn_ExportPreQuantizedLayer.

Computes: out = fake_quant(x) @ dequant(weight_q).T + bias
  x_q  = clip(round_half_away(x / a_scale) + a_zp, 0, 255)
  x_dq = (x_q - a_zp) * a_scale
  W    = (weight_q - w_zp[:, None]) * w_scale[:, None]      # [out, in]
  out  = einsum('bsk,ok->bso', x_dq, W) + bias

Sharding: 2D grid over the 8 cores — 4 shards of out_features (O) x
2 shards of tokens (N).  Each core computes a [2048, 2048] block of
out^T.  Key algebra: with xi = x_q - a_zp and wi = wq - wzp (both exact
small integers, representable in bf16),

  out[o, n] = s * ws[o] * (wi @ xi^T)[o, n] + bias[o]

so the matmul runs at full bf16 PE rate and the epilogue is a single
per-partition (per-o) scale+bias.

Schedule: the x fake-quant uses the HW's round+saturate u8 cast
(f32->u8, alternating between ACT and DVE per k-tile) followed by a
zp-subtract to bf16 (a ~0.16us DVE fast-path op), instead of the fp32
magic-number trick.  x is produced in two N-halves: all 16 k-tiles of
cols 0-1023 first (interleaved with the weight tiles, dequantized as
wq + (-wzp) on the DVE), then cols 1024-2047.  kt0 is emitted as a
minimal dependency chain (w tile + first 512 quantized columns + first
512 dequantized weight columns) so the first matmul issues ~3us in.
The matmul loop walks (n-half, ot) groups of two 512-wide PSUM banks,
so four output groups are in flight in the 8 PSUM banks and the PE
starts consuming k-tiles as they are quantized; the producer cadence
(~1.6-2.2us/k-tile across DMA/ACT/DVE) roughly matches the PE's
1.74us/k-tile appetite, so the PE stays fed through the lead-in.
Epilogues drain each group's two banks on ACT and DVE in parallel and
leave through the Activation HWDGE ring so stores never queue behind
input loads.
"""

import sys

if "/opt/trn_rl_repo" not in sys.path:
    sys.path.insert(0, "/opt/trn_rl_repo")

import ml_dtypes
import numpy as np

import concourse.bass as bass
import concourse.mybir as mybir
import concourse.tile as tile
from concourse import bacc
from concourse.bass_utils import run_bass_kernel_spmd

F32 = mybir.dt.float32
BF16 = mybir.dt.bfloat16
U8 = mybir.dt.uint8

# Full problem shape (hardcoded per spec)
B, S, DIN, DOUT = 2, 2048, 2048, 8192
N_CORES = 8
O_SPLIT, N_SPLIT = 4, 2  # 4 shards of DOUT x 2 shards of tokens

WPATH = "dve"  # "swdge": preset+accum-DMA dequant; "dve": sync DMA + tensor_add
# (measured: swdge is ~20us/rep slower end-to-end despite freeing the DVE)


HINT = False
WARMUP = False
OPOOL_BUFS = 4
KSPLIT = 4  # lead groups that accumulate only kt0-7 first (0 disables)
STAGGER = False  # staggered_reset on the reps-loop back-edge


def build_nc(K, N, O, reps=1, quant="cast", wsub_engine="split", wpath=None,
             hint=None, warmup=None, **_):
    """Build the per-core Bass program.

    reps > 1 wraps the whole body in a device-side loop — used only for
    timing (device work scales with reps while dispatch overhead doesn't).

    Inputs (per core):
      xT      [K, N]   f32   x^T shard (tokens on the free axis)
      wqT     [K, O]   u8    weight_q^T shard
      wzpb    [128, O] bf16  w_zp broadcast along partitions
      aparams [128, 2] f32   (a_scale, a_zp) broadcast along partitions
      wsc     [128, O//128] f32  w_scale laid out [p, ot] with o = ot*128+p
      biasc   [128, O//128] f32  bias, same layout
    Output:
      out     [O, N]  f32    out^T shard
    """
    KT = K // 128
    OT = O // 128
    NH = N // 2

    wpath = WPATH if wpath is None else wpath
    hint = HINT if hint is None else hint
    warmup = WARMUP if warmup is None else warmup
    nc = bacc.Bacc("TRN2", target_bir_lowering=False, debug=False, num_devices=N_CORES)
    xT = nc.declare_dram_parameter("xT", [K, N], F32, isOutput=False)
    wqT = nc.declare_dram_parameter("wqT", [K, O], U8, isOutput=False)
    wzpb = nc.declare_dram_parameter("wzpb", [128, O], BF16, isOutput=False)
    aparams = nc.declare_dram_parameter("aparams", [128, 2], F32, isOutput=False)
    wsc = nc.declare_dram_parameter("wsc", [128, OT], F32, isOutput=False)
    biasc = nc.declare_dram_parameter("biasc", [128, OT], F32, isOutput=False)
    out = nc.declare_dram_parameter("out", [O, N], F32, isOutput=True)

    with tile.TileContext(nc) as tc:
        with (
            tc.tile_pool(name="const", bufs=1) as cpool,
            tc.tile_pool(name="big", bufs=1) as bigpool,
            tc.tile_pool(name="xin", bufs=3) as xpool,
            tc.tile_pool(name="tq", bufs=3) as tpool,
            tc.tile_pool(name="win", bufs=2) as wpool,
            tc.tile_pool(name="oout", bufs=OPOOL_BUFS) as opool,
            tc.tile_pool(name="psum", bufs=8, space="PSUM") as psum_pool,
        ):
            def body():
                _kernel_body(
                    nc, tc, KT, OT, NH, N, O,
                    xT, wqT, wzpb, aparams, wsc, biasc, out,
                    cpool, bigpool, xpool, tpool, wpool, opool, psum_pool,
                    wpath, warmup,
                )

            if reps > 1:
                # PE body is ~2k instructions (>1 IRAM block): hint the
                # back-edge so the branch target prefetches (~3-4us/rep
                # I$-miss otherwise)
                hints = (mybir.EngineType.PE,) if hint else ()
                with tc.For_i(0, reps, 1, hint_engines=hints,
                              staggered_reset=STAGGER):
                    body()
            else:
                body()

    nc.compile()
    return nc


def _kernel_body(
    nc, tc, KT, OT, NH, N, O,
    xT, wqT, wzpb, aparams, wsc, biasc, out,
    cpool, bigpool, xpool, tpool, wpool, opool, psum_pool,
    wpath="swdge", warmup=True,
):
    AF = mybir.ActivationFunctionType
    OP = mybir.AluOpType
    # --- scalar prep -------------------------------------------------
    ap_sb = cpool.tile([128, 2], F32)
    nc.sync.dma_start(ap_sb[:], aparams[:])
    rs = cpool.tile([128, 1], F32)
    nc.vector.reciprocal(rs[:], ap_sb[:, 0:1])
    negz = cpool.tile([128, 1], F32)
    nc.vector.tensor_scalar(negz[:], ap_sb[:, 1:2], -1.0, 0.0, OP.mult, OP.add)

    # --- streaming quantization of x and weight dequant --------------
    # HW-measured op costs drive the engine split (per [128,1024]):
    # f32->u8 quant is 2.2us on ACT / 1.6us on DVE; the u8->bf16 zp-subtract
    # and bf16 preset hit the DVE fast path (~0.16/0.29us per tile).  The
    # weight dequant runs entirely in the DMA path: wiT is preset to -wzp
    # (DVE, 0.29us/tile) and the u8 weights land via a SWDGE dma with
    # accum_op=add (dtype-converting, own queue) — no DVE/GpSimd pass and no
    # qSP traffic.  The f32->u8 cast alternates ACT/DVE per k-tile, so the
    # lead cadence (~1.6us) stays below the PE's 1.74us/k-tile appetite.
    xiT = bigpool.tile([128, KT, N], BF16)
    wiT = bigpool.tile([128, KT, O], BF16)
    nwzpb_sb = cpool.tile([128, O], BF16)
    ws_sb = cpool.tile([128, OT], F32)
    beta = cpool.tile([128, OT], F32)
    alpha = cpool.tile([128, OT], F32)  # alpha = a_scale * w_scale

    def quant(dst, src, engine):
        # xq = sat_u8(round(x*rs + z)): the store-path cast rounds to
        # nearest and saturates at [0,255] = clip(round(x/s) + z, 0, 255)
        xq = tpool.tile([128, src.shape[-1]], U8, name="xq", tag="xq")
        if engine == "act":
            nc.scalar.activation(xq[:], src, AF.Identity,
                                 bias=ap_sb[:, 1:2], scale=rs[:, 0:1])
        else:
            nc.vector.tensor_scalar(xq[:], src, rs[:, 0:1], ap_sb[:, 1:2],
                                    OP.mult, OP.add)
        nc.vector.tensor_scalar_add(dst, xq[:], negz[:, 0:1])

    # HAM keep-warm: ~3.5us of matmuls on the previous rep's (still
    # resident) kt15 tiles, so the PE clock-gate stays at 8/8 through the
    # rep boundary + lead-in (the idle would otherwise re-throttle it to
    # half rate for the first ~3.4us of real matmuls every rep)
    if warmup:
        warm_ps = psum_pool.tile([128, 512], F32, name="warm_ps", tag="psum")
        for _ in range(16):
            nc.tensor.matmul(
                warm_ps[:], wiT[:, KT - 1, 0:128], xiT[:, KT - 1, 0:512],
                start=True, stop=True,
            )

    # kt0: minimal chain to the first real matmul (groups ot0-3 need
    # wi[:, 0:512] and xi[:, 0:512] only)
    ks = slice(0, 128)
    wq_sb = wpool.tile([128, O], U8, name="wq", tag="wq")
    nc.sync.dma_start(wq_sb[:], wqT[ks, :])
    nc.sync.dma_start(nwzpb_sb[:, 0:512], wzpb[:, 0:512])
    nc.vector.tensor_add(wiT[:, 0, 0:512], wq_sb[:, 0:512], nwzpb_sb[:, 0:512])
    xf = xpool.tile([128, NH], F32, name="xf", tag="xf")
    for ci in range(2):
        cs = slice(ci * 512, (ci + 1) * 512)
        nc.sync.dma_start(xf[:, cs], xT[ks, cs])
        quant(xiT[:, 0, cs], xf[:, cs], "act" if ci == 0 else "dve")
    nc.sync.dma_start(nwzpb_sb[:, 512:], wzpb[:, 512:])
    nc.vector.tensor_add(wiT[:, 0, 512:], wq_sb[:, 512:], nwzpb_sb[:, 512:])
    nc.sync.dma_start(ws_sb[:], wsc[:])
    nc.sync.dma_start(beta[:], biasc[:])
    nc.vector.tensor_scalar_mul(alpha[:], ws_sb[:], ap_sb[:, 0:1])

    # n-half 0 of x, interleaved with the weights
    for kt in range(1, KT):
        ks = slice(kt * 128, (kt + 1) * 128)
        xf = xpool.tile([128, NH], F32, name="xf", tag="xf")
        nc.sync.dma_start(xf[:], xT[ks, 0:NH])
        quant(xiT[:, kt, 0:NH], xf[:], "act" if kt % 2 == 0 else "dve")
        if wpath == "swdge":
            # weight dequant in the DMA path: preset -wzp, then accum-add wq
            nc.vector.tensor_scalar_add(wiT[:, kt, :], nwzpb_sb[:], 0.0)
            nc.gpsimd.dma_start(wiT[:, kt, :], wqT[ks, :], accum_op=OP.add)
        else:
            wq_sb = wpool.tile([128, O], U8, name="wq", tag="wq")
            nc.sync.dma_start(wq_sb[:], wqT[ks, :])
            # nwzpb is negated, so dequant is wq + (-wzp)
            nc.vector.tensor_add(wiT[:, kt, :], wq_sb[:], nwzpb_sb[:])

    # n-half 1 of x
    for kt in range(KT):
        ks = slice(kt * 128, (kt + 1) * 128)
        xf = xpool.tile([128, NH], F32, name="xf2", tag="xf2")
        nc.sync.dma_start(xf[:], xT[ks, NH:N])
        quant(xiT[:, kt, NH:N], xf[:], "act" if kt % 2 == 0 else "dve")

    # --- matmul + epilogue -------------------------------------------
    # The 8 PSUM banks cap the PE at 4 in-flight (ot, n-half) groups, i.e.
    # 1.74us of matmul work per arriving k-tile during the lead.  To let the
    # PE catch up once it falls behind, the FIRST `SPL` groups span only
    # kt0-7: their banks free as soon as k-tile 7 lands, the next groups
    # then chew through the already-arrived tiles, and the deferred kt8-15
    # halves run later (combined with the stashed halves by one fused DVE
    # op per 512-block).
    SPL = KSPLIT
    KH = KT // 2
    accs = [cpool.tile([128, NH], F32, name=f"acc{i}") for i in range(SPL)]

    def mm_group(ot, nh, k0, k1, ps):
        for kt in range(k0, k1):
            for j in range(2):
                nc.tensor.matmul(
                    ps[j][:],
                    wiT[:, kt, ot * 128 : (ot + 1) * 128],
                    xiT[:, kt, nh * NH + j * 512 : nh * NH + (j + 1) * 512],
                    start=(kt == k0),
                    stop=(kt == k1 - 1),
                )

    def epilogue(dst, ps, ot):
        nc.scalar.activation(
            dst[:, 0:512], ps[0][:], AF.Identity,
            bias=beta[:, ot : ot + 1], scale=alpha[:, ot : ot + 1],
        )
        nc.vector.tensor_scalar(
            dst[:, 512:1024], ps[1][:],
            alpha[:, ot : ot + 1], beta[:, ot : ot + 1], OP.mult, OP.add,
        )

    def store(ot, nh, osb):
        # outputs leave via the Act HWDGE ring; input loads own the SP ring
        nc.scalar.dma_start(
            out[ot * 128 : (ot + 1) * 128, nh * NH : (nh + 1) * NH], osb[:]
        )

    def psum_pair(tag):
        return [
            psum_pool.tile([128, 512], F32, name=f"ps_{tag}_{j}", tag="psum")
            for j in range(2)
        ]

    # nh0 phase a: k-half A of the first SPL groups -> acc (epilogue applied)
    for ot in range(SPL):
        ps = psum_pair(f"a{ot}")
        mm_group(ot, 0, 0, KH, ps)
        epilogue(accs[ot], ps, ot)
    # nh0 phase b: remaining groups, full K
    for ot in range(SPL, OT):
        ps = psum_pair(f"b{ot}")
        mm_group(ot, 0, 0, KT, ps)
        osb = opool.tile([128, NH], F32, name="osb", tag="osb")
        epilogue(osb, ps, ot)
        store(ot, 0, osb)
    # nh0 phase c: deferred k-half B of the first SPL groups + combine
    for ot in range(SPL):
        ps = psum_pair(f"c{ot}")
        mm_group(ot, 0, KH, KT, ps)
        osb = opool.tile([128, NH], F32, name="osb", tag="osb")
        for j in range(2):
            cs = slice(j * 512, (j + 1) * 512)
            nc.vector.scalar_tensor_tensor(
                osb[:, cs], ps[j][:], alpha[:, ot : ot + 1], accs[ot][:, cs],
                OP.mult, OP.add,
            )
        store(ot, 0, osb)
    # nh1: normal full-K groups
    for ot in range(OT):
        ps = psum_pair(f"d{ot}")
        mm_group(ot, 1, 0, KT, ps)
        osb = opool.tile([128, NH], F32, name="osb", tag="osb")
        epilogue(osb, ps, ot)
        store(ot, 1, osb)


def prep_core_inputs(x, a_scale, a_zp, weight_q, w_scale, w_zp, bias):
    """Host-side sharding/layout: returns the per-core input maps."""
    x = np.asarray(x, dtype=np.float32)
    ntok = x.size // x.shape[-1]
    K = x.shape[-1]
    O_total = weight_q.shape[0]
    Oc = O_total // O_SPLIT
    Nc = ntok // N_SPLIT
    OTc = Oc // 128

    xT = np.ascontiguousarray(x.reshape(ntok, K).T)  # [K, ntok]
    s = np.float32(np.asarray(a_scale).reshape(-1)[0])
    z = np.float32(np.asarray(a_zp).reshape(-1)[0])
    aparams = np.ascontiguousarray(
        np.broadcast_to(np.array([s, z], np.float32), (128, 2))
    )

    x_halves = [
        np.ascontiguousarray(xT[:, i * Nc : (i + 1) * Nc]) for i in range(N_SPLIT)
    ]

    in_maps = []
    for c in range(O_SPLIT * N_SPLIT):
        oc, ncs = divmod(c, N_SPLIT)
        osl = slice(oc * Oc, (oc + 1) * Oc)
        wq_sh = np.asarray(weight_q[osl], dtype=np.uint8)  # values 0..255, lossless
        wqT = np.ascontiguousarray(wq_sh.T)  # [K, Oc]
        # negated: the kernel presets wiT to -wzp and accum-adds wq via DMA
        wzp_sh = (-np.asarray(w_zp[osl], dtype=np.float32)).astype(ml_dtypes.bfloat16)
        wzpb = np.ascontiguousarray(np.broadcast_to(wzp_sh[None, :], (128, Oc)))
        wsc = np.ascontiguousarray(
            np.asarray(w_scale[osl], np.float32).reshape(OTc, 128).T
        )
        biasc = np.ascontiguousarray(
            np.asarray(bias[osl], np.float32).reshape(OTc, 128).T
        )
        in_maps.append(
            {
                "xT": x_halves[ncs],
                "wqT": wqT,
                "wzpb": wzpb,
                "aparams": aparams,
                "wsc": wsc,
                "biasc": biasc,
            }
        )
    return in_maps


_NC_CACHE = {}

QUANT_MODE = "cast"
WSUB_ENGINE = "split"


def _get_nc(K, N, O):
    key = (K, N, O)
    if key not in _NC_CACHE:
        _NC_CACHE[key] = build_nc(K, N, O)
    return _NC_CACHE[key]


def kernel(x, a_scale, a_zp, weight_q, w_scale, w_zp, bias):
    x = np.asarray(x)
    b, seq, K = x.shape
    ntok = b * seq
    O_total = weight_q.shape[0]
    Oc = O_total // O_SPLIT
    Nc = ntok // N_SPLIT

    nc = _get_nc(K, Nc, Oc)
    in_maps = prep_core_inputs(x, a_scale, a_zp, weight_q, w_scale, w_zp, bias)
    res = run_bass_kernel_spmd(nc, in_maps, list(range(N_CORES)))

    outT = np.empty((O_total, ntok), np.float32)
    for c in range(N_CORES):
        oc, ncs = divmod(c, N_SPLIT)
        outT[oc * Oc : (oc + 1) * Oc, ncs * Nc : (ncs + 1) * Nc] = res.results[c]["out"]
    return np.ascontiguousarray(outT.T).reshape(b, seq, O_total)


# revision 18
# speedup vs baseline: 1.1494x; 1.1494x over previous
"""Trainium2 Bass kernel for nn_ExportPreQuantizedLayer.

Computes: out = fake_quant(x) @ dequant(weight_q).T + bias
  x_q  = clip(round_half_away(x / a_scale) + a_zp, 0, 255)
  x_dq = (x_q - a_zp) * a_scale
  W    = (weight_q - w_zp[:, None]) * w_scale[:, None]      # [out, in]
  out  = einsum('bsk,ok->bso', x_dq, W) + bias

Sharding: 2D grid over the 8 cores — 4 shards of out_features (O) x
2 shards of tokens (N).  Each core computes a [2048, 2048] block of
out^T.  Key algebra: with xi = x_q - a_zp and wi = wq - wzp (both exact
small integers, representable in bf16),

  out[o, n] = s * ws[o] * (wi @ xi^T)[o, n] + bias[o]

so the matmul runs at full bf16 PE rate and the epilogue is a single
per-partition (per-o) scale+bias.

Schedule: the x fake-quant uses the HW's round+saturate u8 cast
(f32->u8, alternating between ACT and DVE per k-tile) followed by a
zp-subtract to bf16 (a ~0.16us DVE fast-path op), instead of the fp32
magic-number trick.  x is produced in two N-halves: all 16 k-tiles of
cols 0-1023 first (interleaved with the weight tiles, dequantized as
wq + (-wzp) on the DVE), then cols 1024-2047.  kt0 is emitted as a
minimal dependency chain (w tile + first 512 quantized columns + first
512 dequantized weight columns) so the first matmul issues ~3us in.
The matmul loop walks (n-half, ot) groups of two 512-wide PSUM banks,
so four output groups are in flight in the 8 PSUM banks and the PE
starts consuming k-tiles as they are quantized; the producer cadence
(~1.6-2.2us/k-tile across DMA/ACT/DVE) roughly matches the PE's
1.74us/k-tile appetite, so the PE stays fed through the lead-in.
Epilogues drain each group's two banks on ACT and DVE in parallel and
leave through the Activation HWDGE ring so stores never queue behind
input loads.
"""

import sys

if "/opt/trn_rl_repo" not in sys.path:
    sys.path.insert(0, "/opt/trn_rl_repo")

import ml_dtypes
import numpy as np

import concourse.bass as bass
import concourse.mybir as mybir
import concourse.tile as tile
from concourse import bacc
from concourse.bass_utils import run_bass_kernel_spmd

F32 = mybir.dt.float32
BF16 = mybir.dt.bfloat16
U8 = mybir.dt.uint8

# Full problem shape (hardcoded per spec)
B, S, DIN, DOUT = 2, 2048, 2048, 8192
N_CORES = 8
O_SPLIT, N_SPLIT = 4, 2  # 4 shards of DOUT x 2 shards of tokens

WPATH = "dve"  # "swdge": preset+accum-DMA dequant; "dve": sync DMA + tensor_add
# (measured: swdge is ~20us/rep slower end-to-end despite freeing the DVE)


HINT = False
WARMUP = False
OPOOL_BUFS = 4
KSPLIT = 4  # lead groups that accumulate only kt0-7 first (0 disables)
STAGGER = False  # staggered_reset on the reps-loop back-edge (NaN race - keep off)
XBUFS = 3   # xf/xq staging pool depth
WQ_ENG = "scalar"  # weight loads ride the Act HWDGE ring: x then streams qSP uninterrupted during the lead (measured ~25us/rep faster than sharing qSP)


def build_nc(K, N, O, reps=1, quant="cast", wsub_engine="split", wpath=None,
             hint=None, warmup=None, **_):
    """Build the per-core Bass program.

    reps > 1 wraps the whole body in a device-side loop — used only for
    timing (device work scales with reps while dispatch overhead doesn't).

    Inputs (per core):
      xT      [K, N]   f32   x^T shard (tokens on the free axis)
      wqT     [K, O]   u8    weight_q^T shard
      wzpb    [128, O] bf16  w_zp broadcast along partitions
      aparams [128, 2] f32   (a_scale, a_zp) broadcast along partitions
      wsc     [128, O//128] f32  w_scale laid out [p, ot] with o = ot*128+p
      biasc   [128, O//128] f32  bias, same layout
    Output:
      out     [O, N]  f32    out^T shard
    """
    KT = K // 128
    OT = O // 128
    NH = N // 2

    wpath = WPATH if wpath is None else wpath
    hint = HINT if hint is None else hint
    warmup = WARMUP if warmup is None else warmup
    nc = bacc.Bacc("TRN2", target_bir_lowering=False, debug=False, num_devices=N_CORES)
    xT = nc.declare_dram_parameter("xT", [K, N], F32, isOutput=False)
    wqT = nc.declare_dram_parameter("wqT", [K, O], U8, isOutput=False)
    wzpb = nc.declare_dram_parameter("wzpb", [128, O], BF16, isOutput=False)
    aparams = nc.declare_dram_parameter("aparams", [128, 2], F32, isOutput=False)
    wsc = nc.declare_dram_parameter("wsc", [128, OT], F32, isOutput=False)
    biasc = nc.declare_dram_parameter("biasc", [128, OT], F32, isOutput=False)
    out = nc.declare_dram_parameter("out", [O, N], F32, isOutput=True)

    with tile.TileContext(nc) as tc:
        with (
            tc.tile_pool(name="const", bufs=1) as cpool,
            tc.tile_pool(name="big", bufs=1) as bigpool,
            tc.tile_pool(name="xin", bufs=XBUFS) as xpool,
            tc.tile_pool(name="tq", bufs=XBUFS) as tpool,
            tc.tile_pool(name="win", bufs=2) as wpool,
            tc.tile_pool(name="oout", bufs=OPOOL_BUFS) as opool,
            tc.tile_pool(name="psum", bufs=8, space="PSUM") as psum_pool,
        ):
            def body():
                _kernel_body(
                    nc, tc, KT, OT, NH, N, O,
                    xT, wqT, wzpb, aparams, wsc, biasc, out,
                    cpool, bigpool, xpool, tpool, wpool, opool, psum_pool,
                    wpath, warmup,
                )

            if reps > 1:
                # PE body is ~2k instructions (>1 IRAM block): hint the
                # back-edge so the branch target prefetches (~3-4us/rep
                # I$-miss otherwise)
                hints = (mybir.EngineType.PE,) if hint else ()
                with tc.For_i(0, reps, 1, hint_engines=hints,
                              staggered_reset=STAGGER):
                    body()
            else:
                body()

    nc.compile()
    return nc


def _kernel_body(
    nc, tc, KT, OT, NH, N, O,
    xT, wqT, wzpb, aparams, wsc, biasc, out,
    cpool, bigpool, xpool, tpool, wpool, opool, psum_pool,
    wpath="swdge", warmup=True,
):
    AF = mybir.ActivationFunctionType
    OP = mybir.AluOpType
    # --- scalar prep -------------------------------------------------
    ap_sb = cpool.tile([128, 2], F32)
    nc.sync.dma_start(ap_sb[:], aparams[:])
    rs = cpool.tile([128, 1], F32)
    nc.vector.reciprocal(rs[:], ap_sb[:, 0:1])
    negz = cpool.tile([128, 1], F32)
    nc.vector.tensor_scalar(negz[:], ap_sb[:, 1:2], -1.0, 0.0, OP.mult, OP.add)

    # --- streaming quantization of x and weight dequant --------------
    # HW-measured op costs drive the engine split (per [128,1024]):
    # f32->u8 quant is 2.2us on ACT / 1.6us on DVE; the u8->bf16 zp-subtract
    # and bf16 preset hit the DVE fast path (~0.16/0.29us per tile).  The
    # weight dequant runs entirely in the DMA path: wiT is preset to -wzp
    # (DVE, 0.29us/tile) and the u8 weights land via a SWDGE dma with
    # accum_op=add (dtype-converting, own queue) — no DVE/GpSimd pass and no
    # qSP traffic.  The f32->u8 cast alternates ACT/DVE per k-tile, so the
    # lead cadence (~1.6us) stays below the PE's 1.74us/k-tile appetite.
    xiT = bigpool.tile([128, KT, N], BF16)
    wiT = bigpool.tile([128, KT, O], BF16)
    nwzpb_sb = cpool.tile([128, O], BF16)
    ws_sb = cpool.tile([128, OT], F32)
    beta = cpool.tile([128, OT], F32)
    alpha = cpool.tile([128, OT], F32)  # alpha = a_scale * w_scale

    def quant(dst, src, engine):
        # xq = sat_u8(round(x*rs + z)): the store-path cast rounds to
        # nearest and saturates at [0,255] = clip(round(x/s) + z, 0, 255)
        xq = tpool.tile([128, src.shape[-1]], U8, name="xq", tag="xq")
        if engine == "act":
            nc.scalar.activation(xq[:], src, AF.Identity,
                                 bias=ap_sb[:, 1:2], scale=rs[:, 0:1])
        else:
            nc.vector.tensor_scalar(xq[:], src, rs[:, 0:1], ap_sb[:, 1:2],
                                    OP.mult, OP.add)
        nc.vector.tensor_scalar_add(dst, xq[:], negz[:, 0:1])

    # HAM keep-warm: ~3.5us of matmuls on the previous rep's (still
    # resident) kt15 tiles, so the PE clock-gate stays at 8/8 through the
    # rep boundary + lead-in (the idle would otherwise re-throttle it to
    # half rate for the first ~3.4us of real matmuls every rep)
    if warmup:
        warm_ps = psum_pool.tile([128, 512], F32, name="warm_ps", tag="psum")
        for _ in range(16):
            nc.tensor.matmul(
                warm_ps[:], wiT[:, KT - 1, 0:128], xiT[:, KT - 1, 0:512],
                start=True, stop=True,
            )

    # kt0: minimal chain to the first real matmul (groups ot0-3 need
    # wi[:, 0:512] and xi[:, 0:512] only)
    ks = slice(0, 128)
    wq_sb = wpool.tile([128, O], U8, name="wq", tag="wq")
    getattr(nc, WQ_ENG).dma_start(wq_sb[:], wqT[ks, :])
    nc.sync.dma_start(nwzpb_sb[:, 0:512], wzpb[:, 0:512])
    nc.vector.tensor_add(wiT[:, 0, 0:512], wq_sb[:, 0:512], nwzpb_sb[:, 0:512])
    xf = xpool.tile([128, NH], F32, name="xf", tag="xf")
    for ci in range(2):
        cs = slice(ci * 512, (ci + 1) * 512)
        nc.sync.dma_start(xf[:, cs], xT[ks, cs])
        quant(xiT[:, 0, cs], xf[:, cs], "act" if ci == 0 else "dve")
    nc.sync.dma_start(nwzpb_sb[:, 512:], wzpb[:, 512:])
    nc.vector.tensor_add(wiT[:, 0, 512:], wq_sb[:, 512:], nwzpb_sb[:, 512:])
    nc.sync.dma_start(ws_sb[:], wsc[:])
    nc.sync.dma_start(beta[:], biasc[:])
    nc.vector.tensor_scalar_mul(alpha[:], ws_sb[:], ap_sb[:, 0:1])

    # n-half 0 of x, interleaved with the weights
    for kt in range(1, KT):
        ks = slice(kt * 128, (kt + 1) * 128)
        xf = xpool.tile([128, NH], F32, name="xf", tag="xf")
        nc.sync.dma_start(xf[:], xT[ks, 0:NH])
        quant(xiT[:, kt, 0:NH], xf[:], "act" if kt % 2 == 0 else "dve")
        if wpath == "swdge":
            # weight dequant in the DMA path: preset -wzp, then accum-add wq
            nc.vector.tensor_scalar_add(wiT[:, kt, :], nwzpb_sb[:], 0.0)
            nc.gpsimd.dma_start(wiT[:, kt, :], wqT[ks, :], accum_op=OP.add)
        else:
            wq_sb = wpool.tile([128, O], U8, name="wq", tag="wq")
            getattr(nc, WQ_ENG).dma_start(wq_sb[:], wqT[ks, :])
            # nwzpb is negated, so dequant is wq + (-wzp)
            nc.vector.tensor_add(wiT[:, kt, :], wq_sb[:], nwzpb_sb[:])

    # n-half 1 of x
    for kt in range(KT):
        ks = slice(kt * 128, (kt + 1) * 128)
        xf = xpool.tile([128, NH], F32, name="xf2", tag="xf2")
        nc.sync.dma_start(xf[:], xT[ks, NH:N])
        quant(xiT[:, kt, NH:N], xf[:], "act" if kt % 2 == 0 else "dve")

    # --- matmul + epilogue -------------------------------------------
    # The 8 PSUM banks cap the PE at 4 in-flight (ot, n-half) groups, i.e.
    # 1.74us of matmul work per arriving k-tile during the lead.  To let the
    # PE catch up once it falls behind, the FIRST `SPL` groups span only
    # kt0-7: their banks free as soon as k-tile 7 lands, the next groups
    # then chew through the already-arrived tiles, and the deferred kt8-15
    # halves run later (combined with the stashed halves by one fused DVE
    # op per 512-block).
    SPL = KSPLIT
    KH = KT // 2
    accs = [cpool.tile([128, NH], F32, name=f"acc{i}") for i in range(SPL)]

    def mm_group(ot, nh, k0, k1, ps):
        for kt in range(k0, k1):
            for j in range(2):
                nc.tensor.matmul(
                    ps[j][:],
                    wiT[:, kt, ot * 128 : (ot + 1) * 128],
                    xiT[:, kt, nh * NH + j * 512 : nh * NH + (j + 1) * 512],
                    start=(kt == k0),
                    stop=(kt == k1 - 1),
                )

    def epilogue(dst, ps, ot):
        nc.scalar.activation(
            dst[:, 0:512], ps[0][:], AF.Identity,
            bias=beta[:, ot : ot + 1], scale=alpha[:, ot : ot + 1],
        )
        nc.vector.tensor_scalar(
            dst[:, 512:1024], ps[1][:],
            alpha[:, ot : ot + 1], beta[:, ot : ot + 1], OP.mult, OP.add,
        )

    def store(ot, nh, osb):
        # outputs leave via the Act HWDGE ring; input loads own the SP ring
        nc.scalar.dma_start(
            out[ot * 128 : (ot + 1) * 128, nh * NH : (nh + 1) * NH], osb[:]
        )

    def psum_pair(tag):
        return [
            psum_pool.tile([128, 512], F32, name=f"ps_{tag}_{j}", tag="psum")
            for j in range(2)
        ]

    # nh0 phase a: k-half A of the first SPL groups -> acc (epilogue applied)
    for ot in range(SPL):
        ps = psum_pair(f"a{ot}")
        mm_group(ot, 0, 0, KH, ps)
        epilogue(accs[ot], ps, ot)
    # nh0 phase b: remaining groups, full K
    for ot in range(SPL, OT):
        ps = psum_pair(f"b{ot}")
        mm_group(ot, 0, 0, KT, ps)
        osb = opool.tile([128, NH], F32, name="osb", tag="osb")
        epilogue(osb, ps, ot)
        store(ot, 0, osb)
    # nh0 phase c: deferred k-half B of the first SPL groups + combine
    for ot in range(SPL):
        ps = psum_pair(f"c{ot}")
        mm_group(ot, 0, KH, KT, ps)
        osb = opool.tile([128, NH], F32, name="osb", tag="osb")
        for j in range(2):
            cs = slice(j * 512, (j + 1) * 512)
            nc.vector.scalar_tensor_tensor(
                osb[:, cs], ps[j][:], alpha[:, ot : ot + 1], accs[ot][:, cs],
                OP.mult, OP.add,
            )
        store(ot, 0, osb)
    # nh1: normal full-K groups
    for ot in range(OT):
        ps = psum_pair(f"d{ot}")
        mm_group(ot, 1, 0, KT, ps)
        osb = opool.tile([128, NH], F32, name="osb", tag="osb")
        epilogue(osb, ps, ot)
        store(ot, 1, osb)


def prep_core_inputs(x, a_scale, a_zp, weight_q, w_scale, w_zp, bias):
    """Host-side sharding/layout: returns the per-core input maps."""
    x = np.asarray(x, dtype=np.float32)
    ntok = x.size // x.shape[-1]
    K = x.shape[-1]
    O_total = weight_q.shape[0]
    Oc = O_total // O_SPLIT
    Nc = ntok // N_SPLIT
    OTc = Oc // 128

    xT = np.ascontiguousarray(x.reshape(ntok, K).T)  # [K, ntok]
    s = np.float32(np.asarray(a_scale).reshape(-1)[0])
    z = np.float32(np.asarray(a_zp).reshape(-1)[0])
    aparams = np.ascontiguousarray(
        np.broadcast_to(np.array([s, z], np.float32), (128, 2))
    )

    x_halves = [
        np.ascontiguousarray(xT[:, i * Nc : (i + 1) * Nc]) for i in range(N_SPLIT)
    ]

    in_maps = []
    for c in range(O_SPLIT * N_SPLIT):
        oc, ncs = divmod(c, N_SPLIT)
        osl = slice(oc * Oc, (oc + 1) * Oc)
        wq_sh = np.asarray(weight_q[osl], dtype=np.uint8)  # values 0..255, lossless
        wqT = np.ascontiguousarray(wq_sh.T)  # [K, Oc]
        # negated: the kernel presets wiT to -wzp and accum-adds wq via DMA
        wzp_sh = (-np.asarray(w_zp[osl], dtype=np.float32)).astype(ml_dtypes.bfloat16)
        wzpb = np.ascontiguousarray(np.broadcast_to(wzp_sh[None, :], (128, Oc)))
        wsc = np.ascontiguousarray(
            np.asarray(w_scale[osl], np.float32).reshape(OTc, 128).T
        )
        biasc = np.ascontiguousarray(
            np.asarray(bias[osl], np.float32).reshape(OTc, 128).T
        )
        in_maps.append(
            {
                "xT": x_halves[ncs],
                "wqT": wqT,
                "wzpb": wzpb,
                "aparams": aparams,
                "wsc": wsc,
                "biasc": biasc,
            }
        )
    return in_maps


_NC_CACHE = {}

QUANT_MODE = "cast"
WSUB_ENGINE = "split"


def _get_nc(K, N, O):
    key = (K, N, O)
    if key not in _NC_CACHE:
        _NC_CACHE[key] = build_nc(K, N, O)
    return _NC_CACHE[key]


def kernel(x, a_scale, a_zp, weight_q, w_scale, w_zp, bias):
    x = np.asarray(x)
    b, seq, K = x.shape
    ntok = b * seq
    O_total = weight_q.shape[0]
    Oc = O_total // O_SPLIT
    Nc = ntok // N_SPLIT

    nc = _get_nc(K, Nc, Oc)
    in_maps = prep_core_inputs(x, a_scale, a_zp, weight_q, w_scale, w_zp, bias)
    res = run_bass_kernel_spmd(nc, in_maps, list(range(N_CORES)))

    outT = np.empty((O_total, ntok), np.float32)
    for c in range(N_CORES):
        oc, ncs = divmod(c, N_SPLIT)
        outT[oc * Oc : (oc + 1) * Oc, ncs * Nc : (ncs + 1) * Nc] = res.results[c]["out"]
    return np.ascontiguousarray(outT.T).reshape(b, seq, O_total)
